# revision 5
# baseline (speedup 1.0000x reference)
"""Trainium2 Bass kernel for nn_HConstructor10 (segment_reduce).

Strategy: shard the N=20000 nodes across 8 NeuronCores (2500 nodes/core,
padded to 2560 = 20*128).  Each core runs the full pipeline on its node
shard (t parallel linears -> backbone MLP -> logits -> per-row argmax ->
local class histogram H).  The per-hyperedge feature sums (mask.T @ af2)
and the softmax denominators (column sums of exp(H)) are partial-summed
locally and combined with a single fused 2.1MB AllReduce, after which each
core produces its shard of H_soft and dots.

All activations are kept in "transposed" layout (feature dim on SBUF
partitions, rows on the free axis) so every matmul streams with N=512
moving free dim; logits are produced row-major so the DVE max/max_index
ops give per-row argmax directly.
"""

import os
import sys

for _p in ("/opt/trn_rl_repo", "/root/.axon_site/_ro/trn_rl_repo"):
    if os.path.isdir(_p) and _p not in sys.path:
        sys.path.insert(0, _p)

import numpy as np

import concourse.bass as bass
import concourse.mybir as mybir
import concourse.tile as tile
from concourse import bacc, bass_utils
from concourse.alu_op_type import AluOpType
from concourse.masks import make_identity

F32 = mybir.dt.float32
BF16 = mybir.dt.bfloat16
U32 = mybir.dt.uint32
I32 = mybir.dt.int32
AF = mybir.ActivationFunctionType

# Problem constants (hardcoded per spec).
N, D, NS, T = 20000, 512, 1024, 4
M = 8            # cores
B = T + 1        # replication blocks
NL = N // M      # 2500 local nodes per core
NP = 2560        # padded local nodes (20 * 128)
NT = NP // 128   # 20 node tiles
RC = NP // 512   # 5 row chunks of 512
KD = D // 128    # 4 contraction tiles
SCALE = float(D) ** -0.5

_CACHE = {}


def _build(use_b1, use_b1b, sim=False):
    nc = bacc.Bacc("TRN2", target_bir_lowering=False, debug=False,
                   num_devices=1 if sim else M)
    nc._sim_mode = sim

    feat = nc.dram_tensor("x_features", [NL, D], F32, kind="ExternalInput").ap()
    w_lin = nc.dram_tensor("x_w_lin", [T, D, D], F32, kind="ExternalInput").ap()
    b_lin = nc.dram_tensor("x_b_lin", [T, D], F32, kind="ExternalInput").ap()
    w_bb0 = nc.dram_tensor("x_w_bb0", [D, D], F32, kind="ExternalInput").ap()
    b_bb0 = nc.dram_tensor("x_b_bb0", [D], F32, kind="ExternalInput").ap()
    w_bb1 = nc.dram_tensor("x_w_bb1", [D, D], F32, kind="ExternalInput").ap()
    b_bb1 = nc.dram_tensor("x_b_bb1", [D], F32, kind="ExternalInput").ap()
    w1 = nc.dram_tensor("x_w1", [D, NS], F32, kind="ExternalInput").ap()
    b1 = nc.dram_tensor("x_b1", [NS], F32, kind="ExternalInput").ap()

    hsoft_out = nc.dram_tensor("o_hsoft", [NL, NS], F32, kind="ExternalOutput").ap()
    heT_out = nc.dram_tensor("o_heT", [D, NS], F32, kind="ExternalOutput").ap()
    dots_out = nc.dram_tensor("o_dots", [B, NL, NS], F32, kind="ExternalOutput").ap()

    with tile.TileContext(nc) as tc:
        _body(nc, tc, feat, w_lin, b_lin, w_bb0, b_bb0, w_bb1, b_bb1, w1, b1,
              hsoft_out, heT_out, dots_out, use_b1, use_b1b)
    nc.compile()
    return nc


def _body(nc, tc, feat, w_lin, b_lin, w_bb0, b_bb0, w_bb1, b_bb1, w1, b1,
          hsoft_out, heT_out, dots_out, use_b1, use_b1b):
    dma = nc.sync.dma_start

    with tc.tile_pool(name="dram", bufs=1, space="DRAM") as dp, \
         tc.tile_pool(name="const", bufs=1) as cp:
        # DRAM scratch
        afT_sp = dp.tile([KD, 128, B * NP], F32)     # af^T spill (d-tile, p, row)
        af2r_sp = dp.tile([NT, 128, D], F32)         # block-0 af2 row-major spill
        ar_in = dp.tile([D + 1, NS], F32)            # he^T partial + colsum row
        ar_out = dp.tile([D + 1, NS], F32, addr_space="Shared")
        den_sp = dp.tile([NS], F32)

        # constants
        ident = cp.tile([128, 128], F32)
        make_identity(nc, ident)
        classes_sb = cp.tile([128, B, NT], U32)
        ones2 = cp.tile([128, 2], F32)
        nc.gpsimd.memset(ones2[:], 1.0)
        pidx_i = cp.tile([128, 1], I32)
        nc.gpsimd.iota(pidx_i[:], pattern=[[1, 1]], base=0, channel_multiplier=1)
        pidx_f = cp.tile([128, 1], F32)
        nc.vector.tensor_copy(pidx_f[:], pidx_i[:])
        # ones2[:, 1] = 1.0 for valid rows of the last (partial) node tile
        nc.vector.tensor_scalar(ones2[:, 1:2], pidx_f[:], float(NL % 128), None,
                                AluOpType.is_lt)
        bl_sb = cp.tile([128, T, KD], F32)
        dma(bl_sb[:], b_lin.rearrange("t (mi p) -> p t mi", p=128))
        b0_sb = cp.tile([128, KD], F32)
        dma(b0_sb[:], b_bb0.rearrange("(mi p) -> p mi", p=128))
        b1b_sb = cp.tile([128, KD], F32)
        dma(b1b_sb[:], b_bb1.rearrange("(mi p) -> p mi", p=128))
        if use_b1b:
            b1b_bc = cp.tile([128, D], F32)
            dma(b1b_bc[:], b_bb1.partition_broadcast(128))
        if use_b1:
            b1_bc = cp.tile([128, NS], F32)
            dma(b1_bc[:], b1.partition_broadcast(128))
        iota_f = cp.tile([128, NS], F32)

        # ---------------- Phase B: per-shard pipeline up to classes --------
        with tc.tile_pool(name="wp", bufs=1) as wp, \
             tc.tile_pool(name="wk", bufs=2) as wk, \
             tc.tile_pool(name="ps512", bufs=4, space="PSUM") as ps512, \
             tc.tile_pool(name="ps1k", bufs=2, space="PSUM") as ps1k:
            wl_sb = [wp.tile([128, KD, D], F32, name=f"wl{t}") for t in range(T)]
            for t in range(T):
                dma(wl_sb[t][:], w_lin[t].rearrange("(ki p) m -> p ki m", p=128))
            w0_sb = wp.tile([128, KD, D], F32)
            dma(w0_sb[:], w_bb0.rearrange("(ki p) m -> p ki m", p=128))
            w1b_sb = wp.tile([128, KD, D], F32)
            dma(w1b_sb[:], w_bb1.rearrange("(ki p) m -> p ki m", p=128))
            w1_sb = wp.tile([128, KD, NS], F32)
            dma(w1_sb[:], w1.rearrange("(ki p) m -> p ki m", p=128))

            for rc in range(RC):
                r0 = rc * 512  # node offset within padded block
                # --- block 0: transpose features into af^T layout ---
                tps = [ps512.tile([128, 512], F32, tag="ps", name=f"tp{rc}_{ki}")
                       for ki in range(KD)]
                for jn in range(4):
                    j = rc * 4 + jn
                    frm = wk.tile([128, D], F32, tag="frm", bufs=3)
                    lo = j * 128
                    valid = min(128, NL - lo)
                    if valid < 128:
                        nc.gpsimd.memset(frm[:], 0.0)
                    dma(frm[:valid, :], feat[lo:lo + valid, :])
                    for ki in range(KD):
                        nc.tensor.transpose(
                            tps[ki][:, jn * 128:(jn + 1) * 128],
                            frm[:, ki * 128:(ki + 1) * 128], ident[:])
                aft0 = []
                for ki in range(KD):
                    a = wk.tile([128, 512], F32, tag=f"aft0_{ki}", bufs=2)
                    nc.scalar.copy(a[:], tps[ki][:])
                    aft0.append(a)

                for tb in range(B):
                    # --- af^T chunk for this block ---
                    if tb == 0:
                        afc = aft0
                    else:
                        afc = []
                        for mi in range(KD):
                            ps = ps512.tile([128, 512], F32, tag="ps",
                                            name=f"tr{rc}_{tb}_{mi}")
                            for ki in range(KD):
                                nc.tensor.matmul(
                                    ps[:],
                                    wl_sb[tb - 1][:, ki, mi * 128:(mi + 1) * 128],
                                    aft0[ki][:],
                                    start=(ki == 0), stop=(ki == KD - 1))
                            a = wk.tile([128, 512], F32, tag=f"afc_{mi}", bufs=2)
                            nc.scalar.activation(a[:], ps[:], AF.Identity,
                                                 bias=bl_sb[:, tb - 1, mi:mi + 1])
                            afc.append(a)
                    # spill raw af^T + relu
                    raf = []
                    for mi in range(KD):
                        dma(afT_sp[mi, :, tb * NP + r0: tb * NP + r0 + 512],
                            afc[mi][:])
                        r = wk.tile([128, 512], F32, tag=f"raf_{mi}", bufs=2)
                        nc.vector.tensor_relu(r[:], afc[mi][:])
                        raf.append(r)
                    # --- h = relu(af @ W0 + b0) ---
                    rh = []
                    for mi in range(KD):
                        ps = ps512.tile([128, 512], F32, tag="ps",
                                        name=f"h{rc}_{tb}_{mi}")
                        for ki in range(KD):
                            nc.tensor.matmul(
                                ps[:], w0_sb[:, ki, mi * 128:(mi + 1) * 128],
                                raf[ki][:], start=(ki == 0), stop=(ki == KD - 1))
                        r = wk.tile([128, 512], F32, tag=f"rh_{mi}", bufs=2)
                        nc.scalar.activation(r[:], ps[:], AF.Relu,
                                             bias=b0_sb[:, mi:mi + 1])
                        rh.append(r)
                    # --- af2 = relu(h) @ W1b + b1b (store relu'd, transposed) ---
                    raf2 = []
                    for mi in range(KD):
                        ps = ps512.tile([128, 512], F32, tag="ps",
                                        name=f"a2{rc}_{tb}_{mi}")
                        for ki in range(KD):
                            nc.tensor.matmul(
                                ps[:], w1b_sb[:, ki, mi * 128:(mi + 1) * 128],
                                rh[ki][:], start=(ki == 0), stop=(ki == KD - 1))
                        r = wk.tile([128, 512], F32, tag=f"raf2_{mi}", bufs=2)
                        nc.scalar.activation(r[:], ps[:], AF.Relu,
                                             bias=b1b_sb[:, mi:mi + 1])
                        raf2.append(r)
                    # --- block 0 extra: raw af2 row-major for the he matmul ---
                    if tb == 0:
                        for jn in range(4):
                            j = rc * 4 + jn
                            ps = ps512.tile([128, 512], F32, tag="ps",
                                            name=f"row{rc}_{jn}")
                            for ki in range(KD):
                                nc.tensor.matmul(
                                    ps[:], rh[ki][:, jn * 128:(jn + 1) * 128],
                                    w1b_sb[:, ki, :],
                                    start=(ki == 0), stop=(ki == KD - 1))
                            a2r = wk.tile([128, 512], F32, tag="a2r", bufs=3)
                            if use_b1b:
                                nc.vector.tensor_tensor(a2r[:], ps[:], b1b_bc[:],
                                                        AluOpType.add)
                            else:
                                nc.scalar.copy(a2r[:], ps[:])
                            lo = j * 128
                            valid = min(128, NL - lo)
                            if valid < 128:
                                # zero pad rows via per-partition mask multiply
                                nc.vector.tensor_scalar(
                                    a2r[:], a2r[:], ones2[:, 1:2], None,
                                    AluOpType.mult)
                            dma(af2r_sp[j], a2r[:])
                    # --- logits (row-major) + argmax ---
                    for jn in range(4):
                        j = rc * 4 + jn
                        ps = ps1k.tile([128, NS], F32, tag="psl",
                                       name=f"lg{rc}_{tb}_{jn}")
                        for c2 in range(2):
                            for ki in range(KD):
                                nc.tensor.matmul(
                                    ps[:, c2 * 512:(c2 + 1) * 512],
                                    raf2[ki][:, jn * 128:(jn + 1) * 128],
                                    w1_sb[:, ki, c2 * 512:(c2 + 1) * 512],
                                    start=(ki == 0), stop=(ki == KD - 1))
                        lg = wk.tile([128, NS], F32, tag="lg", bufs=3)
                        if use_b1:
                            nc.vector.tensor_tensor(lg[:], ps[:], b1_bc[:],
                                                    AluOpType.add)
                        else:
                            nc.scalar.copy(lg[:], ps[:])
                        mx8 = wk.tile([128, 8], F32, tag="mx8", bufs=3)
                        cl8 = wk.tile([128, 8], U32, tag="cl8", bufs=3)
                        nc.vector.max(mx8[:], lg[:])
                        nc.vector.max_index(cl8[:], mx8[:], lg[:])
                        nc.vector.tensor_copy(classes_sb[:, tb, j:j + 1],
                                              cl8[:, 0:1])

        # ---------------- Phase C: histogram H, exp colsum ------------------
        iota_i = None
        with tc.tile_pool(name="hp", bufs=1) as hp, \
             tc.tile_pool(name="cw", bufs=2) as cw:
            it = cw.tile([128, NS], I32, tag="ioi")
            nc.gpsimd.iota(it[:], pattern=[[1, NS]], base=0, channel_multiplier=0)
            nc.vector.tensor_copy(iota_f[:], it[:])

            htiles = [hp.tile([128, NS], BF16, name=f"H_{j}") for j in range(NT)]
            with tc.tile_pool(name="psc", bufs=1, space="PSUM") as psc:
                cs_ps = [psc.tile([1, 512], F32, name=f"cs{c2}") for c2 in range(2)]
                for j in range(NT):
                    clsf = cw.tile([128, B], F32, tag="clsf", bufs=2)
                    nc.vector.tensor_copy(clsf[:], classes_sb[:, :, j])
                    H = htiles[j]
                    nc.vector.tensor_scalar(H[:], iota_f[:], clsf[:, 0:1], None,
                                            AluOpType.is_equal)
                    for tb in range(1, B):
                        nc.vector.scalar_tensor_tensor(
                            H[:], iota_f[:], clsf[:, tb:tb + 1], H[:],
                            AluOpType.is_equal, AluOpType.add)
                    ex = cw.tile([128, NS], F32, tag="ex", bufs=2)
                    nc.scalar.activation(ex[:], H[:], AF.Exp)
                    oc = 1 if j == NT - 1 else 0
                    for c2 in range(2):
                        nc.tensor.matmul(cs_ps[c2][:], ones2[:, oc:oc + 1],
                                         ex[:, c2 * 512:(c2 + 1) * 512],
                                         start=(j == 0), stop=(j == NT - 1))
                csum = cw.tile([1, NS], F32, tag="csum")
                for c2 in range(2):
                    nc.scalar.copy(csum[:, c2 * 512:(c2 + 1) * 512], cs_ps[c2][:])
                dma(ar_in[D:D + 1, :], csum[:])

            # ------------- Phase D: partial he^T = af2row^T-contract -------
            with tc.tile_pool(name="psd", bufs=1, space="PSUM") as psd:
                he_ps = [psd.tile([128, 512], F32, name=f"he{di}_{c2}")
                         for di in range(KD) for c2 in range(2)]
                for j in range(NT):
                    a2r = cw.tile([128, D], F32, tag="a2rl", bufs=3)
                    dma(a2r[:], af2r_sp[j])
                    mw = cw.tile([128, NS], F32, tag="mw", bufs=2)
                    nc.vector.tensor_scalar(mw[:], htiles[j][:], 0.5, None,
                                            AluOpType.is_ge)
                    for di in range(KD):
                        for c2 in range(2):
                            nc.tensor.matmul(
                                he_ps[di * 2 + c2][:],
                                a2r[:, di * 128:(di + 1) * 128],
                                mw[:, c2 * 512:(c2 + 1) * 512],
                                start=(j == 0), stop=(j == NT - 1))
                for di in range(KD):
                    for c2 in range(2):
                        hesb = cw.tile([128, 512], F32, tag="hesb", bufs=3)
                        nc.scalar.copy(hesb[:], he_ps[di * 2 + c2][:])
                        dma(ar_in[di * 128:(di + 1) * 128,
                                  c2 * 512:(c2 + 1) * 512], hesb[:])

            # ------------- AllReduce (he^T partial + colsum, fused) --------
            if getattr(nc, "_sim_mode", False):
                dma(ar_out[:], ar_in[:])
            else:
                nc.gpsimd.collective_compute(
                    "AllReduce", AluOpType.add,
                    replica_groups=[list(range(M))],
                    ins=[ar_in.opt()], outs=[ar_out.opt()])

            # ------------- Phase E: H_soft shard + hf^T ---------------------
            dma(heT_out[:], ar_out[0:D, :])
            den = cw.tile([1, NS], F32, tag="den")
            dma(den[:], ar_out[D:D + 1, :])
            rec = cw.tile([1, NS], F32, tag="rec")
            nc.vector.reciprocal(rec[:], den[:])
            dma(den_sp[:], rec[:])
            recb = cp.tile([128, NS], F32)
            dma(recb[:], den_sp.partition_broadcast(128))
            for j in range(NT):
                ex = cw.tile([128, NS], F32, tag="ex2", bufs=2)
                nc.scalar.activation(ex[:], htiles[j][:], AF.Exp)
                hs = cw.tile([128, NS], F32, tag="hs", bufs=3)
                nc.vector.tensor_tensor(hs[:], ex[:], recb[:], AluOpType.mult)
                lo = j * 128
                valid = min(128, NL - lo)
                dma(hsoft_out[lo:lo + valid, :], hs[:valid, :])

        # ---------------- Phase F: dots = af @ (scale*hf)^T -----------------
        with tc.tile_pool(name="fp", bufs=1) as fp, \
             tc.tile_pool(name="fw", bufs=2) as fw, \
             tc.tile_pool(name="psf", bufs=3, space="PSUM") as psf:
            hft = []
            for ki in range(KD):
                h0 = fp.tile([128, NS], F32, name=f"hft{ki}")
                dma(h0[:], ar_out[ki * 128:(ki + 1) * 128, :])
                nc.scalar.mul(h0[:], h0[:], SCALE)
                hft.append(h0)
            for rt in range(B * NT):
                tb, j = rt // NT, rt % NT
                lhs = []
                for ki in range(KD):
                    lt = fw.tile([128, 128], F32, tag=f"lt{ki}", bufs=3)
                    dma(lt[:], afT_sp[ki, :, rt * 128:(rt + 1) * 128])
                    lhs.append(lt)
                ps = psf.tile([128, NS], F32, tag="psf", name=f"d{rt}")
                for c2 in range(2):
                    for ki in range(KD):
                        nc.tensor.matmul(
                            ps[:, c2 * 512:(c2 + 1) * 512], lhs[ki][:],
                            hft[ki][:, c2 * 512:(c2 + 1) * 512],
                            start=(ki == 0), stop=(ki == KD - 1))
                db = fw.tile([128, NS], F32, tag="db", bufs=3)
                nc.scalar.copy(db[:], ps[:])
                lo = j * 128
                valid = min(128, NL - lo)
                dma(dots_out[tb, lo:lo + valid, :], db[:valid, :])


def _get_program(use_b1, use_b1b):
    key = (use_b1, use_b1b)
    if key not in _CACHE:
        _CACHE[key] = _build(use_b1, use_b1b)
    return _CACHE[key]


def kernel(features, W_lin, b_lin, W_bb0, b_bb0, W_bb1, b_bb1, W1, b1,
           t, num_edges):
    features = np.ascontiguousarray(features, np.float32)
    W_lin = np.ascontiguousarray(W_lin, np.float32)
    b_lin = np.ascontiguousarray(b_lin, np.float32)
    W_bb0 = np.ascontiguousarray(W_bb0, np.float32)
    b_bb0 = np.ascontiguousarray(b_bb0, np.float32)
    W_bb1 = np.ascontiguousarray(W_bb1, np.float32)
    b_bb1 = np.ascontiguousarray(b_bb1, np.float32)
    W1 = np.ascontiguousarray(W1, np.float32)
    b1 = np.ascontiguousarray(b1, np.float32)
    assert int(t) == T and int(num_edges) == NS
    assert features.shape == (N, D)

    use_b1 = bool(np.any(b1))
    use_b1b = bool(np.any(b_bb1))
    nc = _get_program(use_b1, use_b1b)

    in_maps = []
    for c in range(M):
        in_maps.append({
            "x_features": np.ascontiguousarray(features[c * NL:(c + 1) * NL]),
            "x_w_lin": W_lin, "x_b_lin": b_lin,
            "x_w_bb0": W_bb0, "x_b_bb0": b_bb0,
            "x_w_bb1": W_bb1, "x_b_bb1": b_bb1,
            "x_w1": W1, "x_b1": b1,
        })
    res = bass_utils.run_bass_kernel_spmd(nc, in_maps, core_ids=list(range(M)))
    rs = res.results

    h_soft = np.concatenate([rs[c]["o_hsoft"] for c in range(M)], axis=0)
    hf = np.ascontiguousarray(rs[0]["o_heT"].T)
    dots = np.concatenate([rs[c]["o_dots"] for c in range(M)], axis=1)
    dots = np.ascontiguousarray(dots.reshape(B * N, NS))
    return h_soft, hf, dots
